# revision 26
# baseline (speedup 1.0000x reference)
"""Causal self-attention for Trainium2, 8 NeuronCores.

Sharding: tensor-parallel over heads (4 heads/core) x data-parallel over
batch (2). Core i handles batch i//4, heads 4*(i%4)..4*(i%4)+3. Each core
computes its heads' attention output and a partial output projection
(W_proj rows for its heads); the host sums the 4 partials per batch and
adds b_proj.

Device layout choices:
  - Q^T, K^T computed feature-major [dim, t] directly (lhsT = W chunk,
    rhs = x^T chunk), so attention scores come out as S^T [k, q] with k
    on partitions -- which is exactly the layout the P@V matmul needs
    as its rhs. No on-chip transposes of the O(T^2) object.
  - V computed in natural [t, dim] layout (lhsT = x^T chunk, rhs = W_v),
    which is the lhsT layout the P@V matmul needs. A ones-column is
    appended to V so the softmax denominators fall out of the same
    matmul (row 64 of the PSUM output).
  - exp() without max subtraction: scores are q.k/8 with q,k ~ N(0,1),
    bounded well inside fp32 exp range; softmax is shift-invariant so
    the result is mathematically identical to the reference.
  - MM1 score matmul pairs (the two heads of a head-pair) use disjoint
    64-row PE tiles (auto tile_position) and partially overlap in the
    array.
  - Diagonal k-chunks are narrowed: matmuls, exp and masking only touch
    columns the causal mask can reach; fully-masked prefix columns are
    never computed or zeroed (the PV matmul simply never accumulates
    them).
  - exp() owns the Scalar engine; all PSUM->SBUF evacuation stays on
    Vector (GpSimd cannot access PSUM); GpSimd handles SBUF-side DMA
    issue and the softmax-denominator broadcast.
  - Softmax normalization is split into two emission points so the
    in-order Vector queue never blocks on the denominator DMA round
    trip; the one instance on the kernel's tail (last qc, second head
    pair) uses an on-chip partition_broadcast + fast reciprocal for
    minimum latency, and the last qc's output projection is split by
    contraction half so it overlaps that qc's own attention.
  - Input/output DMAs alternate between the two HWDGE queues (sync,
    scalar) + SWDGE (gpsimd) so transfers parallelize across SDMA
    engines and the first matmul starts early.

The causal mask is handled by skipping fully-masked k-chunks and
multiplying exp(S) by a precomputed 0/1 indicator on the 128-column
diagonal-straddling block only. If the runtime mask is not the
lower-tri causal mask, a general fallback multiplies by the actual mask
(DMA'd transposed) instead; an all-ones mask drops masking entirely.
"""

import numpy as np

B, T, C, H = 2, 2048, 1024, 16
D = C // H            # 64 head dim
NCORES = 8
NBG = 2               # batch shards
NHG = 4               # head-group shards
HL = H // NHG         # 4 heads per core
DL = HL * D           # 256 local feature dims
NDQ = DL // 128       # 2 partition chunks of local dims
NTB = T // 512        # 4 t-chunks of 512
NKC = T // 128        # 16 key chunks of 128
NQC = T // 512        # 4 query chunks of 512
NTT = T // 128        # 16 t-tiles of 128 (proj / V)

_CACHE = {}


def _build(mode, debug_dump=False):
    """Build + compile the per-core Bass program. mode: causal|full|general."""
    import concourse.bass as bass
    import concourse.bacc as bacc
    import concourse.tile as tile
    import concourse.mybir as mybir

    f32 = mybir.dt.float32
    bf16 = mybir.dt.bfloat16
    Exp = mybir.ActivationFunctionType.Exp
    mult = mybir.AluOpType.mult
    add = mybir.AluOpType.add

    nc = bacc.Bacc(
        "TRN2", target_bir_lowering=False, debug=False, num_devices=NCORES
    )

    xT = nc.dram_tensor("xT", [C, T], bf16, kind="ExternalInput").ap()
    Wl = nc.dram_tensor("Wl", [C, 3 * DL], bf16, kind="ExternalInput").ap()
    bqk = nc.dram_tensor("bqk", [128, 2 * NDQ], f32, kind="ExternalInput").ap()
    bv = nc.dram_tensor("bv", [1, DL], f32, kind="ExternalInput").ap()
    Wp = nc.dram_tensor("Wp", [DL, C], bf16, kind="ExternalInput").ap()
    maskT = None
    if mode == "general":
        maskT = nc.dram_tensor("maskT", [T, T], bf16, kind="ExternalInput").ap()
    yp = nc.dram_tensor("yp", [T, C], bf16, kind="ExternalOutput").ap()
    dbg = {}
    if debug_dump:
        for nm, shp, dt in [
            ("ot_d", [128, NDQ, T], bf16),
        ]:
            dbg[nm] = nc.dram_tensor(nm, shp, dt, kind="ExternalOutput").ap()

    with tile.TileContext(nc) as tc:
        with (
            tc.tile_pool(name="singles", bufs=1) as singles,
            tc.tile_pool(name="xin", bufs=2) as xin,
            tc.tile_pool(name="ptiles", bufs=6) as ptiles,
            tc.tile_pool(name="small", bufs=4) as small,
            tc.tile_pool(name="outp", bufs=3) as outp,
            tc.tile_pool(name="psum", bufs=7, space="PSUM") as psum,
        ):
            def ps512(name):
                return psum.tile(
                    [128, 512], f32, name="ps512", tag="ps512", bufs=4
                )

            # ---- resident inputs ----
            # W and x loads split per kc-chunk AND per partition-half on
            # the two HWDGE queues so the first matmuls start as soon as
            # their pieces land (per-DMA transfers run on single SDMA
            # engines; halving doubles effective arrival bandwidth).
            W_sb = singles.tile([128, 8, 3 * DL], bf16)
            Wl_r = Wl.rearrange("(kc p) n -> p kc n", p=128)
            x0_sb = xin.tile([128, 8, 512], bf16, tag="x_sb", name="x_sb")
            x0r = xT.rearrange("(kc p) t -> p kc t", p=128)[:, :, 0:512]
            for kc in range(8):
                qa, qb = (
                    (nc.sync, nc.scalar) if kc % 2 == 0
                    else (nc.scalar, nc.sync)
                )
                qa.dma_start(out=W_sb[:, kc, :], in_=Wl_r[:, kc, :])
                qb.dma_start(out=x0_sb[:, kc, :], in_=x0r[:, kc, :])
            bqk_sb = singles.tile([128, 2 * NDQ], f32)
            nc.sync.dma_start(out=bqk_sb, in_=bqk)
            bv_row = singles.tile([1, DL], f32)
            nc.sync.dma_start(out=bv_row, in_=bv)
            bv_sb = singles.tile([128, DL], f32)
            nc.gpsimd.partition_broadcast(bv_sb, bv_row)

            ind = None
            if mode == "causal":
                ind = singles.tile([128, 4, 512], bf16)
                for j in range(4):
                    nc.vector.memset(ind[:, j, :], 1.0)
                    # keep (=1.0) iff f - p - 128*j >= 0, else 0.0
                    nc.gpsimd.affine_select(
                        out=ind[:, j, :],
                        in_=ind[:, j, :],
                        compare_op=mybir.AluOpType.is_ge,
                        fill=0.0,
                        base=-128 * j,
                        pattern=[[1, 512]],
                        channel_multiplier=-1,
                    )

            # ---- resident intermediates ----
            QT = singles.tile([128, NDQ, T], bf16)   # [dim%128, dimchunk, t]
            KT = singles.tile([128, NDQ, T], bf16)
            V1 = singles.tile([128, NKC, HL, D + 1], bf16)  # [t%128, kc, h, d+1]
            nc.vector.memset(V1[:, :, :, D : D + 1], 1.0)
            OT = singles.tile([128, NDQ, T], bf16)
            Wp_sb = singles.tile([128, NDQ, C], bf16)

            # ---- phase 1: QKV projections (as interleavable units) ----
            def p1_units(tb, x_sb):
                """Units for one 512-wide t-chunk of the QKV projection."""
                units = []
                if tb > 0:
                    def dma_u(tb=tb, x_sb=x_sb):
                        xr = xT.rearrange("(kc p) t -> p kc t", p=128)[
                            :, :, tb * 512 : (tb + 1) * 512
                        ]
                        for kc in range(8):
                            nc.sync.dma_start(
                                out=x_sb[:, kc, :], in_=xr[:, kc, :]
                            )
                    units.append(dma_u)
                for s in range(2):  # 0=Q, 1=K
                    for dq in range(NDQ):
                        def qk_u(tb=tb, s=s, dq=dq, x_sb=x_sb):
                            ps = ps512("qk")
                            col = s * DL + dq * 128
                            for kc in range(8):
                                nc.tensor.matmul(
                                    ps,
                                    lhsT=W_sb[:, kc, col : col + 128],
                                    rhs=x_sb[:, kc, :],
                                    start=(kc == 0),
                                    stop=(kc == 7),
                                )
                            dst = (QT if s == 0 else KT)[
                                :, dq, tb * 512 : (tb + 1) * 512
                            ]
                            nc.vector.tensor_scalar_add(
                                dst, ps,
                                bqk_sb[:, s * NDQ + dq : s * NDQ + dq + 1],
                            )
                        units.append(qk_u)
                for t4 in range(4):
                    def v_u(tb=tb, t4=t4, x_sb=x_sb):
                        tt = tb * 4 + t4
                        ps = ps512("v")
                        for kc in range(8):
                            nc.tensor.matmul(
                                ps[:, :DL],
                                lhsT=x_sb[:, kc, t4 * 128 : (t4 + 1) * 128],
                                rhs=W_sb[:, kc, 2 * DL : 3 * DL],
                                start=(kc == 0),
                                stop=(kc == 7),
                            )
                        nc.vector.tensor_tensor(
                            out=V1[:, tt, :, 0:D],
                            in0=ps[:, :DL].rearrange("p (h d) -> p h d", d=D),
                            in1=bv_sb.rearrange("p (h d) -> p h d", d=D),
                            op=add,
                        )
                    units.append(v_u)
                return units

            # ---- phase 2/3 units ----
            def proj_mms(pp, tt, n, dqs, start, stop):
                for dq in dqs:
                    nc.tensor.matmul(
                        pp,
                        lhsT=OT[:, dq, tt * 128 : (tt + 1) * 128],
                        rhs=Wp_sb[:, dq, n * 512 : (n + 1) * 512],
                        start=start and dq == dqs[0],
                        stop=stop and dq == dqs[-1],
                    )

            def proj_out(tt, pps, last=False, tailq=False):
                y_sb = outp.tile([128, C], bf16, name="y_sb")
                for n in range(2):
                    if last:
                        nc.scalar.copy(
                            y_sb[:, n * 512 : (n + 1) * 512], pps[n]
                        )
                    else:
                        nc.vector.tensor_copy(
                            y_sb[:, n * 512 : (n + 1) * 512], pps[n]
                        )
                row = tt * 128
                if last:
                    # drain the final tile fast: 4 pieces on 3 queues
                    for r0, c0, engq in [
                        (0, 0, nc.sync), (64, 0, nc.gpsimd),
                        (0, 512, nc.scalar), (64, 512, nc.sync),
                    ]:
                        engq.dma_start(
                            out=yp[row + r0 : row + r0 + 64, c0 : c0 + 512],
                            in_=y_sb[r0 : r0 + 64, c0 : c0 + 512],
                        )
                else:
                    q1 = nc.scalar if tailq else nc.sync
                    q2 = nc.scalar if tailq else nc.gpsimd
                    q1.dma_start(
                        out=yp[row : row + 128, 0:512], in_=y_sb[:, 0:512]
                    )
                    q2.dma_start(
                        out=yp[row : row + 128, 512:1024],
                        in_=y_sb[:, 512:1024],
                    )

            def proj_units(qc, last_t4=None, tailq=False):
                units = []
                for t4 in range(4):
                    def u(qc=qc, t4=t4, last=None, tailq=tailq):
                        last = (t4 == last_t4) if last is None else last
                        tt = qc * 4 + t4
                        pps = []
                        for n in range(2):
                            pp = ps512("proj")
                            proj_mms(pp, tt, n, [0, 1], True, True)
                            pps.append(pp)
                        proj_out(tt, pps, last=last, tailq=tailq)
                    units.append(u)
                return units

            def proj_split_units(qc, t4s, sc_t4s=(), last_t4=None):
                """(a_unit, b_unit) pairs: a computes the dq=0 half of the
                contraction (needs only hp0's normalized OT rows), b adds
                dq=1 and stores. Lets the last qc's projection overlap its
                own second head-pair's attention. t4s in sc_t4s borrow the
                score pool's PSUM banks (dead by the time they run)."""
                pairs = []
                for t4 in t4s:
                    st = {}
                    def a_u(qc=qc, t4=t4, st=st):
                        tt = qc * 4 + t4
                        if t4 in sc_t4s:
                            pp_pair = psum.tile(
                                [128, 2, 512], f32, name="ps1024",
                                tag="ps1024", bufs=2,
                            )
                            st["pps"] = [pp_pair[:, 0, :], pp_pair[:, 1, :]]
                        else:
                            st["pps"] = []
                            for n in range(2):
                                pp = ps512("proj")
                                st["pps"].append(pp)
                        for n in range(2):
                            proj_mms(st["pps"][n], tt, n, [0], True, False)
                    def b_u(qc=qc, t4=t4, st=st):
                        tt = qc * 4 + t4
                        for n in range(2):
                            proj_mms(st["pps"][n], tt, n, [1], False, True)
                        proj_out(
                            tt, st["pps"], last=(t4 == last_t4), tailq=True
                        )
                    pairs.append((a_u, b_u))
                return pairs

            def attn_units(qc, hp, nkc, m_sb, tail=False):
                """One head-pair's attention over all k-chunks, softmax
                denominators via the ones-column of V1."""
                state = {}

                def emit_mm1(kc):
                    # narrow diagonal chunks to the columns the causal
                    # mask can reach (f >= 128*j)
                    j = kc - 4 * qc
                    lo = 128 * j if (mode == "causal" and j >= 0) else 0
                    stp = psum.tile(
                        [128, 2, 512], f32, name="ps1024", tag="ps1024", bufs=2
                    )
                    for hh in range(2):
                        off = 64 * hh
                        nc.tensor.matmul(
                            stp[:, hh, lo:],
                            lhsT=KT[off : off + 64, hp, kc * 128 : (kc + 1) * 128],
                            rhs=QT[
                                off : off + 64, hp,
                                qc * 512 + lo : (qc + 1) * 512,
                            ],
                            start=True,
                            stop=True,
                        )
                    state.setdefault("st", {})[kc] = stp

                def prologue():
                    state["ops"] = [ps512("o"), ps512("o")]
                    state["stage"] = small.tile(
                        [65, 2, 512], f32, tag="stage", bufs=2, name="stage"
                    )
                    state["emitted"] = min(2, nkc)  # lookahead 1
                    for j in range(state["emitted"]):
                        emit_mm1(j)

                def consume(kc):
                    ops = state["ops"]
                    stp = state["st"].pop(kc)
                    p2 = ptiles.tile([128, 2, 512], bf16, tag="p")
                    j = kc - 4 * qc
                    if mode == "causal" and j >= 0:
                        # exp only the live columns; multiply the single
                        # diagonal-straddling 128-col block by the 0/1
                        # indicator (both heads at once). Columns left of
                        # 128*j are never computed nor read downstream.
                        lo = 128 * j
                        nc.scalar.activation(
                            p2[:, :, lo:], stp[:, :, lo:], Exp
                        )
                        base = ind[:, j, lo : lo + 128]
                        ind2 = bass.AP(
                            tensor=base.tensor,
                            offset=base.offset,
                            ap=[base.ap[0], [0, 2], base.ap[1]],
                        )
                        nc.vector.tensor_tensor(
                            out=p2[:, :, lo : lo + 128],
                            in0=p2[:, :, lo : lo + 128],
                            in1=ind2,
                            op=mult,
                        )
                    else:
                        lo = 0
                        nc.scalar.activation(p2, stp, Exp)
                        if mode == "general":
                            base = m_sb[:, kc, :]
                            msk2 = bass.AP(
                                tensor=base.tensor,
                                offset=base.offset,
                                ap=[base.ap[0], [0, 2], base.ap[1]],
                            )
                            nc.vector.tensor_tensor(
                                out=p2, in0=p2, in1=msk2, op=mult
                            )
                    for hh in range(2):
                        h = hp * 2 + hh
                        nc.tensor.matmul(
                            ops[hh][: D + 1, lo:],
                            lhsT=V1[:, kc, h, :],
                            rhs=p2[:, hh, lo:],
                            start=(kc == 0),
                            stop=(kc == nkc - 1),
                        )
                    if state["emitted"] < nkc:
                        emit_mm1(state["emitted"])
                        state["emitted"] += 1

                def stash():
                    # unnormalized output rows (Vector) + denominators
                    # (GpSimd; engines can't move data across partitions,
                    # so the denominator row stays on partition 64 and a
                    # DMA gathers it)
                    ops = state["ops"]
                    for hh in range(2):
                        off = 64 * hh
                        if tail and hh == 0:
                            # scalar engine is done with exp by now; let it
                            # pull one denominator row while Vector does
                            # the other (parallel engines shorten the tail)
                            nc.scalar.copy(
                                state["stage"][64:65, hh, :],
                                ops[hh][D : D + 1, :],
                            )
                        else:
                            nc.vector.tensor_copy(
                                state["stage"][64:65, hh, :],
                                ops[hh][D : D + 1, :],
                            )
                        nc.vector.tensor_copy(
                            OT[off : off + 64, hp, qc * 512 : (qc + 1) * 512],
                            ops[hh][0:D, :],
                        )

                units = [prologue]
                for kc in range(nkc):
                    units.append(lambda kc=kc: consume(kc))
                units.append(stash)
                return units, state

            rcp_dram = nc.dram_tensor(
                "rcp_scratch", [NQC, NDQ, NDQ, 512], f32, kind="Internal"
            ).ap()

            def norm_a(qc, hp, stage, st, tail=False):
                # per-(qc, head-pair) normalization part 1. Two variants:
                # cheap (DVE reciprocal on [2,512] + DRAM-bounce broadcast,
                # latency fully hidden by the schedule) for all but the
                # last instance; latency-optimal (on-chip broadcast of the
                # raw denominators + reciprocal) for the one instance that
                # sits on the kernel's tail.
                if tail:
                    pb = small.tile([1, NDQ, 512], f32, tag="pb", bufs=1)
                    nc.sync.dma_start(out=pb, in_=stage[64:65, :, :])
                    den_b = small.tile(
                        [128, NDQ, 512], f32, tag="den_b", bufs=1
                    )
                    nc.gpsimd.partition_broadcast(den_b, pb)
                    rb_hp = small.tile(
                        [128, NDQ, 512], f32, tag="rbw", bufs=1
                    )
                    nc.vector.reciprocal_approx_fast(out=rb_hp, in_=den_b)
                    st["rbw"] = rb_hp
                    return
                sums2 = small.tile([NDQ, 512], f32, tag="sums2", bufs=2)
                nc.sync.dma_start(out=sums2, in_=stage[64:65, :, :])
                rcp2 = small.tile([NDQ, 512], f32, tag="rcp2", bufs=2)
                nc.vector.reciprocal_approx_fast(out=rcp2, in_=sums2)
                nc.sync.dma_start(out=rcp_dram[qc, hp], in_=rcp2)
                rb_hp = small.tile([128, 512], f32, tag="rb", bufs=2)
                for hh in range(2):
                    bsrc = rcp_dram[qc, hp, hh : hh + 1, :]
                    bsrc = bass.AP(
                        tensor=bsrc.tensor,
                        offset=bsrc.offset,
                        ap=[[0, 64], bsrc.ap[-1]],
                    )
                    nc.gpsimd.dma_start(
                        out=rb_hp[64 * hh : 64 * hh + 64, :], in_=bsrc
                    )
                st["rb"] = rb_hp

            def norm_b_slice(qc, hp, st, t4):
                c0 = qc * 512 + t4 * 128
                for hh in range(2):
                    off = 64 * hh
                    nc.vector.tensor_tensor(
                        out=OT[off : off + 64, hp, c0 : c0 + 128],
                        in0=OT[off : off + 64, hp, c0 : c0 + 128],
                        in1=st["rbw"][
                            off : off + 64, hh, t4 * 128 : (t4 + 1) * 128
                        ],
                        op=mult,
                    )

            def norm_b(qc, hp, st):
                if "rbw" in st:
                    for t4 in range(4):
                        norm_b_slice(qc, hp, st, t4)
                    return
                nc.vector.tensor_tensor(
                    out=OT[:, hp, qc * 512 : (qc + 1) * 512],
                    in0=OT[:, hp, qc * 512 : (qc + 1) * 512],
                    in1=st["rb"],
                    op=mult,
                )

            # ---- schedule: staircase interleave ----
            # attn(qc) needs phase-1 chunks tb <= qc only, so phase-1(tb+1)
            # and proj(qc-1) units are injected between attention units to
            # keep the PE FIFO fed while ACT paces the exp chain.
            def interleave(part, inj, front=3):
                # front-load a couple of injected units right after the
                # first part unit (the prologue) -- that is where the PE
                # would otherwise idle waiting for the first exp
                k, m, j = len(part), len(inj), 0
                for i, u in enumerate(part):
                    u()
                    if i == 0:
                        while j < min(front, m):
                            inj[j]()
                            j += 1
                        continue
                    take = (i + 1) * m // k - i * m // k
                    for _ in range(take):
                        if j < m:
                            inj[j]()
                            j += 1

            # tb0: emit only the dq0 Q/K projections up front -- the
            # first head-pair's attention needs nothing else; dq1 QK and
            # the V units are injected into qc0's schedule so attention
            # starts as soon as the last x0 chunk lands.
            p10 = p1_units(0, x0_sb)
            # layout: [qk(s0,dq0), qk(s0,dq1), qk(s1,dq0), qk(s1,dq1), v*4]
            p10[0]()
            p10[2]()
            qc0_extra = [p10[4], p10[5], p10[1], p10[6], p10[7], p10[3]]
            Wp_r = Wp.rearrange("(dq p) n -> p dq n", p=128)
            nc.scalar.dma_start(out=Wp_sb, in_=Wp_r)
            carry = []  # norm units of the previous qc's hp1
            for qc in range(NQC):
                nkc = 4 * qc + 4 if mode == "causal" else NKC
                m_sb = None
                if mode == "general":
                    m_sb = xin.tile([128, NKC, 512], bf16, tag="mask", bufs=1)
                    nc.sync.dma_start(
                        out=m_sb,
                        in_=maskT.rearrange("(kc p) q -> p kc q", p=128)[
                            :, :, qc * 512 : (qc + 1) * 512
                        ],
                    )
                tail = qc == NQC - 1
                u0, st0 = attn_units(qc, 0, nkc, m_sb)
                u1, st1 = attn_units(qc, 1, nkc, m_sb, tail=tail)
                na0 = lambda qc=qc, st0=st0: norm_a(qc, 0, st0["stage"], st0)
                nb0 = lambda qc=qc, st0=st0: norm_b(qc, 0, st0)
                na1 = lambda qc=qc, st1=st1, tail=tail: norm_a(
                    qc, 1, st1["stage"], st1, tail=tail
                )
                nb1 = lambda qc=qc, st1=st1: norm_b(qc, 1, st1)
                if qc + 1 < NTB:
                    x_next = xin.tile(
                        [128, 8, 512], bf16, tag="x_sb", name="x_sb"
                    )
                    p1 = p1_units(qc + 1, x_next)
                    # V units go to the second half, where injection work
                    # is otherwise thin (esp. qc=0 with no projection yet)
                    inj_early, p1_v = p1[:5], p1[5:]
                else:
                    inj_early, p1_v = [], []
                inj_late = proj_units(qc - 1) if qc >= 1 else []
                if qc == 0:
                    inj_early = qc0_extra + inj_early
                # carry = [na, nb] of the previous qc's hp1: recip early in
                # this half0, multiply at its end (hides the DMA bounce)
                if carry:
                    inj_early = [carry[0]] + inj_early + [carry[1]]
                if qc < NQC - 1:
                    interleave(u0, inj_early)
                    interleave(u1, [na0] + inj_late + p1_v + [nb0])
                    carry = [na1, nb1]
                else:
                    # last qc: proj(qc-1) goes into the first half; the
                    # last qc's own projection starts mid-second-half via
                    # dq-split units so the tail only pays the dq=1 adds.
                    interleave(u0, inj_early + inj_late)
                    pairs = proj_split_units(
                        qc, [0, 1, 2, 3], sc_t4s=(2, 3), last_t4=3
                    )
                    n1 = len(u1)
                    for i, u in enumerate(u1):
                        u()
                        if i == 1:
                            na0()
                        if i == 7:
                            nb0()
                        if i == min(9, n1 - 2):
                            pairs[0][0]()
                        if i == n1 - 1:  # right after stash
                            na1()
                            pairs[2][0]()  # score-bank a-units need no
                            pairs[3][0]()  # stash -> they fill first
                            pairs[1][0]()
                    for t4 in range(4):
                        norm_b_slice(qc, 1, st1, t4)
                        pairs[t4][1]()

            if debug_dump:
                nc.sync.dma_start(out=dbg["ot_d"], in_=OT)

    nc.compile()
    return nc


def _host_prep(x, prefix_causal_mask, W_attn, b_attn, W_proj):
    """Split full inputs into 8 per-core input maps; detect mask mode."""
    scale = 1.0 / np.sqrt(np.float32(D))
    mask = np.asarray(prefix_causal_mask)
    if mask.all():
        mode = "full"
    else:
        tri = np.tril(np.ones((T, T), dtype=bool))
        if all(np.array_equal(mask[b], tri) for b in range(B)):
            mode = "causal"
        else:
            mode = "general"

    import ml_dtypes

    bf16 = ml_dtypes.bfloat16
    x = np.asarray(x, dtype=np.float32)
    W_attn = np.asarray(W_attn, dtype=np.float32)
    b_attn = np.asarray(b_attn, dtype=np.float32)
    W_proj = np.asarray(W_proj, dtype=np.float32)

    in_maps = []
    for core in range(NCORES):
        b = core // NHG
        hg = core % NHG
        lo = hg * DL
        hi = lo + DL
        xT = np.ascontiguousarray(x[b].T)  # [C, T]
        Wq = W_attn[:, lo:hi] * scale
        Wk = W_attn[:, C + lo : C + hi]
        Wv = W_attn[:, 2 * C + lo : 2 * C + hi]
        Wl = np.ascontiguousarray(np.concatenate([Wq, Wk, Wv], axis=1))
        bq = b_attn[lo:hi] * scale
        bk = b_attn[C + lo : C + hi]
        # bias per partition for Q,K chunks: cols = [q0, q1, k0, k1]
        bqk = np.stack(
            [bq[0:128], bq[128:256], bk[0:128], bk[128:256]], axis=1
        ).astype(np.float32)
        bv = np.ascontiguousarray(
            b_attn[2 * C + lo : 2 * C + hi][None, :]
        ).astype(np.float32)
        Wp = np.ascontiguousarray(W_proj[lo:hi, :])
        im = {
            "xT": xT.astype(bf16),
            "Wl": Wl.astype(bf16),
            "bqk": np.ascontiguousarray(bqk),
            "bv": bv,
            "Wp": Wp.astype(bf16),
        }
        if mode == "general":
            im["maskT"] = np.ascontiguousarray(mask[b].T).astype(bf16)
        in_maps.append(im)
    return mode, in_maps


def _get_program(mode):
    if mode not in _CACHE:
        _CACHE[mode] = _build(mode)
    return _CACHE[mode]


def _run(inputs, trace=False):
    """Returns (full_output [B,T,C], BassKernelResults)."""
    from concourse import bass_utils

    mode, in_maps = _host_prep(
        inputs["x"],
        inputs["prefix_causal_mask"],
        inputs["W_attn"],
        inputs["b_attn"],
        inputs["W_proj"],
    )
    nc = _get_program(mode)
    res = bass_utils.run_bass_kernel_spmd(
        nc, in_maps, core_ids=list(range(NCORES)), trace=trace
    )
    b_proj = np.asarray(inputs["b_proj"], dtype=np.float32)
    y = np.zeros((B, T, C), dtype=np.float32)
    for core in range(NCORES):
        y[core // NHG] += np.asarray(res.results[core]["yp"], dtype=np.float32)
    y += b_proj[None, None, :]
    return y, res


def kernel(**inputs):
    y, _ = _run(inputs, trace=False)
    return y


# revision 27
# speedup vs baseline: 1.0020x; 1.0020x over previous
"""Causal self-attention for Trainium2, 8 NeuronCores.

Sharding: tensor-parallel over heads (4 heads/core) x data-parallel over
batch (2). Core i handles batch i//4, heads 4*(i%4)..4*(i%4)+3. Each core
computes its heads' attention output and a partial output projection
(W_proj rows for its heads); the host sums the 4 partials per batch and
adds b_proj.

Device layout choices:
  - Q^T, K^T computed feature-major [dim, t] directly (lhsT = W chunk,
    rhs = x^T chunk), so attention scores come out as S^T [k, q] with k
    on partitions -- which is exactly the layout the P@V matmul needs
    as its rhs. No on-chip transposes of the O(T^2) object.
  - V computed in natural [t, dim] layout (lhsT = x^T chunk, rhs = W_v),
    which is the lhsT layout the P@V matmul needs. A ones-column is
    appended to V so the softmax denominators fall out of the same
    matmul (row 64 of the PSUM output).
  - exp() without max subtraction: scores are q.k/8 with q,k ~ N(0,1),
    bounded well inside fp32 exp range; softmax is shift-invariant so
    the result is mathematically identical to the reference.
  - MM1 score matmul pairs (the two heads of a head-pair) use disjoint
    64-row PE tiles (auto tile_position) and partially overlap in the
    array.
  - Diagonal k-chunks are narrowed: matmuls, exp and masking only touch
    columns the causal mask can reach; fully-masked prefix columns are
    never computed or zeroed (the PV matmul simply never accumulates
    them).
  - exp() owns the Scalar engine; all PSUM->SBUF evacuation stays on
    Vector (GpSimd cannot access PSUM); GpSimd handles SBUF-side DMA
    issue and the softmax-denominator broadcast.
  - Softmax normalization is split into two emission points so the
    in-order Vector queue never blocks on the denominator DMA round
    trip; the one instance on the kernel's tail (last qc, second head
    pair) uses an on-chip partition_broadcast + fast reciprocal for
    minimum latency, and the last qc's output projection is split by
    contraction half so it overlaps that qc's own attention.
  - Input/output DMAs alternate between the two HWDGE queues (sync,
    scalar) + SWDGE (gpsimd) so transfers parallelize across SDMA
    engines and the first matmul starts early.

The causal mask is handled by skipping fully-masked k-chunks and
multiplying exp(S) by a precomputed 0/1 indicator on the 128-column
diagonal-straddling block only. If the runtime mask is not the
lower-tri causal mask, a general fallback multiplies by the actual mask
(DMA'd transposed) instead; an all-ones mask drops masking entirely.
"""

import numpy as np

B, T, C, H = 2, 2048, 1024, 16
D = C // H            # 64 head dim
NCORES = 8
NBG = 2               # batch shards
NHG = 4               # head-group shards
HL = H // NHG         # 4 heads per core
DL = HL * D           # 256 local feature dims
NDQ = DL // 128       # 2 partition chunks of local dims
NTB = T // 512        # 4 t-chunks of 512
NKC = T // 128        # 16 key chunks of 128
NQC = T // 512        # 4 query chunks of 512
NTT = T // 128        # 16 t-tiles of 128 (proj / V)

_CACHE = {}


def _build(mode, debug_dump=False):
    """Build + compile the per-core Bass program. mode: causal|full|general."""
    import concourse.bass as bass
    import concourse.bacc as bacc
    import concourse.tile as tile
    import concourse.mybir as mybir

    f32 = mybir.dt.float32
    bf16 = mybir.dt.bfloat16
    Exp = mybir.ActivationFunctionType.Exp
    mult = mybir.AluOpType.mult
    add = mybir.AluOpType.add

    nc = bacc.Bacc(
        "TRN2", target_bir_lowering=False, debug=False, num_devices=NCORES
    )

    xT = nc.dram_tensor("xT", [C, T], bf16, kind="ExternalInput").ap()
    Wl = nc.dram_tensor("Wl", [C, 3 * DL], bf16, kind="ExternalInput").ap()
    bqk = nc.dram_tensor("bqk", [128, 2 * NDQ], f32, kind="ExternalInput").ap()
    bv = nc.dram_tensor("bv", [1, DL], f32, kind="ExternalInput").ap()
    Wp = nc.dram_tensor("Wp", [DL, C], bf16, kind="ExternalInput").ap()
    maskT = None
    if mode == "general":
        maskT = nc.dram_tensor("maskT", [T, T], bf16, kind="ExternalInput").ap()
    yp = nc.dram_tensor("yp", [T, C], bf16, kind="ExternalOutput").ap()
    dbg = {}
    if debug_dump:
        for nm, shp, dt in [
            ("ot_d", [128, NDQ, T], bf16),
        ]:
            dbg[nm] = nc.dram_tensor(nm, shp, dt, kind="ExternalOutput").ap()

    with tile.TileContext(nc) as tc:
        with (
            tc.tile_pool(name="singles", bufs=1) as singles,
            tc.tile_pool(name="xin", bufs=2) as xin,
            tc.tile_pool(name="ptiles", bufs=6) as ptiles,
            tc.tile_pool(name="small", bufs=4) as small,
            tc.tile_pool(name="outp", bufs=3) as outp,
            tc.tile_pool(name="psum", bufs=7, space="PSUM") as psum,
        ):
            def ps512(name):
                return psum.tile(
                    [128, 512], f32, name="ps512", tag="ps512", bufs=4
                )

            # ---- resident inputs ----
            # W and x loads split per kc-chunk AND per partition-half on
            # the two HWDGE queues so the first matmuls start as soon as
            # their pieces land (per-DMA transfers run on single SDMA
            # engines; halving doubles effective arrival bandwidth).
            W_sb = singles.tile([128, 8, 3 * DL], bf16)
            Wl_r = Wl.rearrange("(kc p) n -> p kc n", p=128)
            x0_sb = xin.tile([128, 8, 512], bf16, tag="x_sb", name="x_sb")
            x0r = xT.rearrange("(kc p) t -> p kc t", p=128)[:, :, 0:512]
            for kc in range(8):
                qa, qb = (
                    (nc.sync, nc.scalar) if kc % 2 == 0
                    else (nc.scalar, nc.sync)
                )
                qa.dma_start(out=W_sb[:, kc, :], in_=Wl_r[:, kc, :])
                qb.dma_start(out=x0_sb[:, kc, :], in_=x0r[:, kc, :])
            bqk_sb = singles.tile([128, 2 * NDQ], f32)
            nc.sync.dma_start(out=bqk_sb, in_=bqk)
            bv_row = singles.tile([1, DL], f32)
            nc.sync.dma_start(out=bv_row, in_=bv)
            bv_sb = singles.tile([128, DL], f32)
            nc.gpsimd.partition_broadcast(bv_sb, bv_row)

            ind = None
            if mode == "causal":
                ind = singles.tile([128, 4, 512], bf16)
                for j in range(4):
                    nc.vector.memset(ind[:, j, :], 1.0)
                    # keep (=1.0) iff f - p - 128*j >= 0, else 0.0
                    nc.gpsimd.affine_select(
                        out=ind[:, j, :],
                        in_=ind[:, j, :],
                        compare_op=mybir.AluOpType.is_ge,
                        fill=0.0,
                        base=-128 * j,
                        pattern=[[1, 512]],
                        channel_multiplier=-1,
                    )

            # ---- resident intermediates ----
            QT = singles.tile([128, NDQ, T], bf16)   # [dim%128, dimchunk, t]
            KT = singles.tile([128, NDQ, T], bf16)
            V1 = singles.tile([128, NKC, HL, D + 1], bf16)  # [t%128, kc, h, d+1]
            nc.vector.memset(V1[:, :, :, D : D + 1], 1.0)
            OT = singles.tile([128, NDQ, T], bf16)
            Wp_sb = singles.tile([128, NDQ, C], bf16)

            # ---- phase 1: QKV projections (as interleavable units) ----
            def p1_units(tb, x_sb):
                """Units for one 512-wide t-chunk of the QKV projection."""
                units = []
                if tb > 0:
                    def dma_u(tb=tb, x_sb=x_sb):
                        xr = xT.rearrange("(kc p) t -> p kc t", p=128)[
                            :, :, tb * 512 : (tb + 1) * 512
                        ]
                        for kc in range(8):
                            nc.sync.dma_start(
                                out=x_sb[:, kc, :], in_=xr[:, kc, :]
                            )
                    units.append(dma_u)
                for s in range(2):  # 0=Q, 1=K
                    for dq in range(NDQ):
                        def qk_u(tb=tb, s=s, dq=dq, x_sb=x_sb):
                            ps = ps512("qk")
                            col = s * DL + dq * 128
                            for kc in range(8):
                                nc.tensor.matmul(
                                    ps,
                                    lhsT=W_sb[:, kc, col : col + 128],
                                    rhs=x_sb[:, kc, :],
                                    start=(kc == 0),
                                    stop=(kc == 7),
                                )
                            dst = (QT if s == 0 else KT)[
                                :, dq, tb * 512 : (tb + 1) * 512
                            ]
                            nc.vector.tensor_scalar_add(
                                dst, ps,
                                bqk_sb[:, s * NDQ + dq : s * NDQ + dq + 1],
                            )
                        units.append(qk_u)
                for t4 in range(4):
                    def v_u(tb=tb, t4=t4, x_sb=x_sb):
                        tt = tb * 4 + t4
                        ps = ps512("v")
                        for kc in range(8):
                            nc.tensor.matmul(
                                ps[:, :DL],
                                lhsT=x_sb[:, kc, t4 * 128 : (t4 + 1) * 128],
                                rhs=W_sb[:, kc, 2 * DL : 3 * DL],
                                start=(kc == 0),
                                stop=(kc == 7),
                            )
                        nc.vector.tensor_tensor(
                            out=V1[:, tt, :, 0:D],
                            in0=ps[:, :DL].rearrange("p (h d) -> p h d", d=D),
                            in1=bv_sb.rearrange("p (h d) -> p h d", d=D),
                            op=add,
                        )
                    units.append(v_u)
                return units

            # ---- phase 2/3 units ----
            def proj_mms(pp, tt, n, dqs, start, stop):
                for dq in dqs:
                    nc.tensor.matmul(
                        pp,
                        lhsT=OT[:, dq, tt * 128 : (tt + 1) * 128],
                        rhs=Wp_sb[:, dq, n * 512 : (n + 1) * 512],
                        start=start and dq == dqs[0],
                        stop=stop and dq == dqs[-1],
                    )

            def proj_out(tt, pps, last=False, tailq=False):
                y_sb = outp.tile([128, C], bf16, name="y_sb")
                for n in range(2):
                    if last:
                        nc.scalar.copy(
                            y_sb[:, n * 512 : (n + 1) * 512], pps[n]
                        )
                    else:
                        nc.vector.tensor_copy(
                            y_sb[:, n * 512 : (n + 1) * 512], pps[n]
                        )
                row = tt * 128
                if last:
                    # drain the final tile fast: 4 pieces on 3 queues
                    for r0, c0, engq in [
                        (0, 0, nc.sync), (64, 0, nc.gpsimd),
                        (0, 512, nc.scalar), (64, 512, nc.sync),
                    ]:
                        engq.dma_start(
                            out=yp[row + r0 : row + r0 + 64, c0 : c0 + 512],
                            in_=y_sb[r0 : r0 + 64, c0 : c0 + 512],
                        )
                else:
                    q1 = nc.scalar if tailq else nc.sync
                    q2 = nc.scalar if tailq else nc.gpsimd
                    q1.dma_start(
                        out=yp[row : row + 128, 0:512], in_=y_sb[:, 0:512]
                    )
                    q2.dma_start(
                        out=yp[row : row + 128, 512:1024],
                        in_=y_sb[:, 512:1024],
                    )

            def proj_units(qc, last_t4=None, tailq=False):
                units = []
                for t4 in range(4):
                    def u(qc=qc, t4=t4, last=None, tailq=tailq):
                        last = (t4 == last_t4) if last is None else last
                        tt = qc * 4 + t4
                        pps = []
                        for n in range(2):
                            pp = ps512("proj")
                            proj_mms(pp, tt, n, [0, 1], True, True)
                            pps.append(pp)
                        proj_out(tt, pps, last=last, tailq=tailq)
                    units.append(u)
                return units

            def proj_split_units(qc, t4s, sc_t4s=(), last_t4=None):
                """(a_unit, b_unit) pairs: a computes the dq=0 half of the
                contraction (needs only hp0's normalized OT rows), b adds
                dq=1 and stores. Lets the last qc's projection overlap its
                own second head-pair's attention. t4s in sc_t4s borrow the
                score pool's PSUM banks (dead by the time they run)."""
                pairs = []
                for t4 in t4s:
                    st = {}
                    def a_u(qc=qc, t4=t4, st=st):
                        tt = qc * 4 + t4
                        if t4 in sc_t4s:
                            pp_pair = psum.tile(
                                [128, 2, 512], f32, name="ps1024",
                                tag="ps1024", bufs=2,
                            )
                            st["pps"] = [pp_pair[:, 0, :], pp_pair[:, 1, :]]
                        else:
                            st["pps"] = []
                            for n in range(2):
                                pp = ps512("proj")
                                st["pps"].append(pp)
                        for n in range(2):
                            proj_mms(st["pps"][n], tt, n, [0], True, False)
                    def b_u(qc=qc, t4=t4, st=st):
                        tt = qc * 4 + t4
                        for n in range(2):
                            proj_mms(st["pps"][n], tt, n, [1], False, True)
                        proj_out(
                            tt, st["pps"], last=(t4 == last_t4), tailq=True
                        )
                    pairs.append((a_u, b_u))
                return pairs

            def attn_units(qc, hp, nkc, m_sb, tail=False):
                """One head-pair's attention over all k-chunks, softmax
                denominators via the ones-column of V1."""
                state = {}

                def emit_mm1(kc):
                    # narrow diagonal chunks to the columns the causal
                    # mask can reach (f >= 128*j)
                    j = kc - 4 * qc
                    lo = 128 * j if (mode == "causal" and j >= 0) else 0
                    stp = psum.tile(
                        [128, 2, 512], f32, name="ps1024", tag="ps1024", bufs=2
                    )
                    for hh in range(2):
                        off = 64 * hh
                        nc.tensor.matmul(
                            stp[:, hh, lo:],
                            lhsT=KT[off : off + 64, hp, kc * 128 : (kc + 1) * 128],
                            rhs=QT[
                                off : off + 64, hp,
                                qc * 512 + lo : (qc + 1) * 512,
                            ],
                            start=True,
                            stop=True,
                        )
                    state.setdefault("st", {})[kc] = stp

                def prologue():
                    state["ops"] = [ps512("o"), ps512("o")]
                    state["stage"] = small.tile(
                        [65, 2, 512], f32, tag="stage", bufs=2, name="stage"
                    )
                    state["emitted"] = min(2, nkc)  # lookahead 1
                    for j in range(state["emitted"]):
                        emit_mm1(j)

                def consume(kc):
                    ops = state["ops"]
                    stp = state["st"].pop(kc)
                    p2 = ptiles.tile([128, 2, 512], bf16, tag="p")
                    j = kc - 4 * qc
                    if mode == "causal" and j >= 0:
                        # exp only the live columns; multiply the single
                        # diagonal-straddling 128-col block by the 0/1
                        # indicator (both heads at once). Columns left of
                        # 128*j are never computed nor read downstream.
                        lo = 128 * j
                        nc.scalar.activation(
                            p2[:, :, lo:], stp[:, :, lo:], Exp
                        )
                        base = ind[:, j, lo : lo + 128]
                        ind2 = bass.AP(
                            tensor=base.tensor,
                            offset=base.offset,
                            ap=[base.ap[0], [0, 2], base.ap[1]],
                        )
                        nc.vector.tensor_tensor(
                            out=p2[:, :, lo : lo + 128],
                            in0=p2[:, :, lo : lo + 128],
                            in1=ind2,
                            op=mult,
                        )
                    else:
                        lo = 0
                        nc.scalar.activation(p2, stp, Exp)
                        if mode == "general":
                            base = m_sb[:, kc, :]
                            msk2 = bass.AP(
                                tensor=base.tensor,
                                offset=base.offset,
                                ap=[base.ap[0], [0, 2], base.ap[1]],
                            )
                            nc.vector.tensor_tensor(
                                out=p2, in0=p2, in1=msk2, op=mult
                            )
                    for hh in range(2):
                        h = hp * 2 + hh
                        nc.tensor.matmul(
                            ops[hh][: D + 1, lo:],
                            lhsT=V1[:, kc, h, :],
                            rhs=p2[:, hh, lo:],
                            start=(kc == 0),
                            stop=(kc == nkc - 1),
                        )
                    if state["emitted"] < nkc:
                        emit_mm1(state["emitted"])
                        state["emitted"] += 1

                def stash():
                    # unnormalized output rows (Vector) + denominators
                    # (GpSimd; engines can't move data across partitions,
                    # so the denominator row stays on partition 64 and a
                    # DMA gathers it)
                    ops = state["ops"]
                    for hh in range(2):
                        off = 64 * hh
                        if tail and hh == 0:
                            # scalar engine is done with exp by now; let it
                            # pull one denominator row while Vector does
                            # the other (parallel engines shorten the tail)
                            nc.scalar.copy(
                                state["stage"][64:65, hh, :],
                                ops[hh][D : D + 1, :],
                            )
                        else:
                            nc.vector.tensor_copy(
                                state["stage"][64:65, hh, :],
                                ops[hh][D : D + 1, :],
                            )
                        nc.vector.tensor_copy(
                            OT[off : off + 64, hp, qc * 512 : (qc + 1) * 512],
                            ops[hh][0:D, :],
                        )

                units = [prologue]
                for kc in range(nkc):
                    units.append(lambda kc=kc: consume(kc))
                units.append(stash)
                return units, state

            rcp_dram = nc.dram_tensor(
                "rcp_scratch", [NQC, NDQ, NDQ, 512], f32, kind="Internal"
            ).ap()

            def norm_a(qc, hp, stage, st, tail=False):
                # per-(qc, head-pair) normalization part 1. Two variants:
                # cheap (DVE reciprocal on [2,512] + DRAM-bounce broadcast,
                # latency fully hidden by the schedule) for all but the
                # last instance; latency-optimal (on-chip broadcast of the
                # raw denominators + reciprocal) for the one instance that
                # sits on the kernel's tail.
                if tail:
                    pb = small.tile([1, NDQ, 512], f32, tag="pb", bufs=1)
                    nc.sync.dma_start(out=pb, in_=stage[64:65, :, :])
                    den_b = small.tile(
                        [128, NDQ, 512], f32, tag="den_b", bufs=1
                    )
                    nc.gpsimd.partition_broadcast(den_b, pb)
                    rb_hp = small.tile(
                        [128, NDQ, 512], f32, tag="rbw", bufs=1
                    )
                    nc.vector.reciprocal_approx_fast(out=rb_hp, in_=den_b)
                    st["rbw"] = rb_hp
                    return
                sums2 = small.tile([NDQ, 512], f32, tag="sums2", bufs=2)
                nc.sync.dma_start(out=sums2, in_=stage[64:65, :, :])
                rcp2 = small.tile([NDQ, 512], f32, tag="rcp2", bufs=2)
                nc.vector.reciprocal_approx_fast(out=rcp2, in_=sums2)
                nc.sync.dma_start(out=rcp_dram[qc, hp], in_=rcp2)
                rb_hp = small.tile([128, 512], f32, tag="rb", bufs=2)
                for hh in range(2):
                    bsrc = rcp_dram[qc, hp, hh : hh + 1, :]
                    bsrc = bass.AP(
                        tensor=bsrc.tensor,
                        offset=bsrc.offset,
                        ap=[[0, 64], bsrc.ap[-1]],
                    )
                    nc.gpsimd.dma_start(
                        out=rb_hp[64 * hh : 64 * hh + 64, :], in_=bsrc
                    )
                st["rb"] = rb_hp

            def norm_b_slice(qc, hp, st, t4):
                c0 = qc * 512 + t4 * 128
                for hh in range(2):
                    off = 64 * hh
                    nc.vector.tensor_tensor(
                        out=OT[off : off + 64, hp, c0 : c0 + 128],
                        in0=OT[off : off + 64, hp, c0 : c0 + 128],
                        in1=st["rbw"][
                            off : off + 64, hh, t4 * 128 : (t4 + 1) * 128
                        ],
                        op=mult,
                    )

            def norm_b(qc, hp, st):
                if "rbw" in st:
                    for t4 in range(4):
                        norm_b_slice(qc, hp, st, t4)
                    return
                nc.vector.tensor_tensor(
                    out=OT[:, hp, qc * 512 : (qc + 1) * 512],
                    in0=OT[:, hp, qc * 512 : (qc + 1) * 512],
                    in1=st["rb"],
                    op=mult,
                )

            # ---- schedule: staircase interleave ----
            # attn(qc) needs phase-1 chunks tb <= qc only, so phase-1(tb+1)
            # and proj(qc-1) units are injected between attention units to
            # keep the PE FIFO fed while ACT paces the exp chain.
            def interleave(part, inj, front=3):
                # front-load a couple of injected units right after the
                # first part unit (the prologue) -- that is where the PE
                # would otherwise idle waiting for the first exp
                k, m, j = len(part), len(inj), 0
                for i, u in enumerate(part):
                    u()
                    if i == 0:
                        while j < min(front, m):
                            inj[j]()
                            j += 1
                        continue
                    take = (i + 1) * m // k - i * m // k
                    for _ in range(take):
                        if j < m:
                            inj[j]()
                            j += 1

            # tb0: emit only the dq0 Q/K projections up front -- the
            # first head-pair's attention needs nothing else; dq1 QK and
            # the V units are injected into qc0's schedule so attention
            # starts as soon as the last x0 chunk lands.
            p10 = p1_units(0, x0_sb)
            # layout: [qk(s0,dq0), qk(s0,dq1), qk(s1,dq0), qk(s1,dq1), v*4]
            # Fused variant of p10[0] + p10[2] (Q and K, dq0): interleave
            # their per-kc accumulations so the arrival-paced startup
            # window feeds two matmuls per landed x0 chunk.
            ps_q = ps512("qk")
            ps_k = ps512("qk")
            for kc in range(8):
                for s, ps in ((0, ps_q), (1, ps_k)):
                    nc.tensor.matmul(
                        ps,
                        lhsT=W_sb[:, kc, s * DL : s * DL + 128],
                        rhs=x0_sb[:, kc, :],
                        start=(kc == 0),
                        stop=(kc == 7),
                    )
            for s, ps in ((0, ps_q), (1, ps_k)):
                nc.vector.tensor_scalar_add(
                    (QT if s == 0 else KT)[:, 0, 0:512], ps,
                    bqk_sb[:, s * NDQ : s * NDQ + 1],
                )
            qc0_extra = [p10[4], p10[5], p10[1], p10[6], p10[7], p10[3]]
            Wp_r = Wp.rearrange("(dq p) n -> p dq n", p=128)
            nc.scalar.dma_start(out=Wp_sb, in_=Wp_r)
            carry = []  # norm units of the previous qc's hp1
            for qc in range(NQC):
                nkc = 4 * qc + 4 if mode == "causal" else NKC
                m_sb = None
                if mode == "general":
                    m_sb = xin.tile([128, NKC, 512], bf16, tag="mask", bufs=1)
                    nc.sync.dma_start(
                        out=m_sb,
                        in_=maskT.rearrange("(kc p) q -> p kc q", p=128)[
                            :, :, qc * 512 : (qc + 1) * 512
                        ],
                    )
                tail = qc == NQC - 1
                u0, st0 = attn_units(qc, 0, nkc, m_sb)
                u1, st1 = attn_units(qc, 1, nkc, m_sb, tail=tail)
                na0 = lambda qc=qc, st0=st0: norm_a(qc, 0, st0["stage"], st0)
                nb0 = lambda qc=qc, st0=st0: norm_b(qc, 0, st0)
                na1 = lambda qc=qc, st1=st1, tail=tail: norm_a(
                    qc, 1, st1["stage"], st1, tail=tail
                )
                nb1 = lambda qc=qc, st1=st1: norm_b(qc, 1, st1)
                if qc + 1 < NTB:
                    x_next = xin.tile(
                        [128, 8, 512], bf16, tag="x_sb", name="x_sb"
                    )
                    p1 = p1_units(qc + 1, x_next)
                    # V units go to the second half, where injection work
                    # is otherwise thin (esp. qc=0 with no projection yet)
                    inj_early, p1_v = p1[:5], p1[5:]
                else:
                    inj_early, p1_v = [], []
                inj_late = proj_units(qc - 1) if qc >= 1 else []
                if qc == 0:
                    inj_early = qc0_extra + inj_early
                # carry = [na, nb] of the previous qc's hp1: recip early in
                # this half0, multiply at its end (hides the DMA bounce)
                if carry:
                    inj_early = [carry[0]] + inj_early + [carry[1]]
                if qc < NQC - 1:
                    interleave(u0, inj_early)
                    interleave(u1, [na0] + inj_late + p1_v + [nb0])
                    carry = [na1, nb1]
                else:
                    # last qc: proj(qc-1) goes into the first half; the
                    # last qc's own projection starts mid-second-half via
                    # dq-split units so the tail only pays the dq=1 adds.
                    interleave(u0, inj_early + inj_late[:-1])
                    pairs = proj_split_units(
                        qc, [0, 1, 2, 3], sc_t4s=(2, 3), last_t4=3
                    )
                    n1 = len(u1)
                    for i, u in enumerate(u1):
                        u()
                        if i == 1:
                            na0()
                        if i == 4:
                            inj_late[-1]()
                        if i == 7:
                            nb0()
                        if i == min(9, n1 - 2):
                            pairs[0][0]()
                        if i == n1 - 1:  # right after stash
                            na1()
                            pairs[2][0]()  # score-bank a-units need no
                            pairs[3][0]()  # stash -> they fill first
                            pairs[1][0]()
                    for t4 in range(4):
                        norm_b_slice(qc, 1, st1, t4)
                        pairs[t4][1]()

            if debug_dump:
                nc.sync.dma_start(out=dbg["ot_d"], in_=OT)

    nc.compile()
    return nc


def _host_prep(x, prefix_causal_mask, W_attn, b_attn, W_proj):
    """Split full inputs into 8 per-core input maps; detect mask mode."""
    scale = 1.0 / np.sqrt(np.float32(D))
    mask = np.asarray(prefix_causal_mask)
    if mask.all():
        mode = "full"
    else:
        tri = np.tril(np.ones((T, T), dtype=bool))
        if all(np.array_equal(mask[b], tri) for b in range(B)):
            mode = "causal"
        else:
            mode = "general"

    import ml_dtypes

    bf16 = ml_dtypes.bfloat16
    x = np.asarray(x, dtype=np.float32)
    W_attn = np.asarray(W_attn, dtype=np.float32)
    b_attn = np.asarray(b_attn, dtype=np.float32)
    W_proj = np.asarray(W_proj, dtype=np.float32)

    in_maps = []
    for core in range(NCORES):
        b = core // NHG
        hg = core % NHG
        lo = hg * DL
        hi = lo + DL
        xT = np.ascontiguousarray(x[b].T)  # [C, T]
        Wq = W_attn[:, lo:hi] * scale
        Wk = W_attn[:, C + lo : C + hi]
        Wv = W_attn[:, 2 * C + lo : 2 * C + hi]
        Wl = np.ascontiguousarray(np.concatenate([Wq, Wk, Wv], axis=1))
        bq = b_attn[lo:hi] * scale
        bk = b_attn[C + lo : C + hi]
        # bias per partition for Q,K chunks: cols = [q0, q1, k0, k1]
        bqk = np.stack(
            [bq[0:128], bq[128:256], bk[0:128], bk[128:256]], axis=1
        ).astype(np.float32)
        bv = np.ascontiguousarray(
            b_attn[2 * C + lo : 2 * C + hi][None, :]
        ).astype(np.float32)
        Wp = np.ascontiguousarray(W_proj[lo:hi, :])
        im = {
            "xT": xT.astype(bf16),
            "Wl": Wl.astype(bf16),
            "bqk": np.ascontiguousarray(bqk),
            "bv": bv,
            "Wp": Wp.astype(bf16),
        }
        if mode == "general":
            im["maskT"] = np.ascontiguousarray(mask[b].T).astype(bf16)
        in_maps.append(im)
    return mode, in_maps


def _get_program(mode):
    if mode not in _CACHE:
        _CACHE[mode] = _build(mode)
    return _CACHE[mode]


def _run(inputs, trace=False):
    """Returns (full_output [B,T,C], BassKernelResults)."""
    from concourse import bass_utils

    mode, in_maps = _host_prep(
        inputs["x"],
        inputs["prefix_causal_mask"],
        inputs["W_attn"],
        inputs["b_attn"],
        inputs["W_proj"],
    )
    nc = _get_program(mode)
    res = bass_utils.run_bass_kernel_spmd(
        nc, in_maps, core_ids=list(range(NCORES)), trace=trace
    )
    b_proj = np.asarray(inputs["b_proj"], dtype=np.float32)
    y = np.zeros((B, T, C), dtype=np.float32)
    for core in range(NCORES):
        y[core // NHG] += np.asarray(res.results[core]["yp"], dtype=np.float32)
    y += b_proj[None, None, :]
    return y, res


def kernel(**inputs):
    y, _ = _run(inputs, trace=False)
    return y
